# revision 33
# baseline (speedup 1.0000x reference)
"""Multi-head attention (B=4, T=2048, C=1024, 16 heads, no mask) on 8 TRN2 cores.

Sharding: batch DP x head TP per the hint. Core c handles batch b=c//2 and
heads 8*(c%2)..8*(c%2)+7 (4 head-pairs) over ALL 2048 queries. Each core
computes a PARTIAL output (its 8 heads through its Wo row-slice); the host
sums the two partials per batch (free w.r.t. HW exec time). No duplicated
K/V work, no DRAM scratch: q/k/v live entirely in SBUF.

All matmul operands bf16 (fp32 PSUM accumulate). Device dataflow per core:
  v[t,d]    = xT.T @ WvT(slice)    -> SBUF aug tiles [128j, 16g, 65] (ones col)
  kT[p]     = WkT(chunk).T @ xT    qT[p] = WqT(chunk).T @ xT   (per pair p)
  per pair p, query-block ib (512), key-tile g (128):
    sAB[j, 2*512] = kT_h.T @ qT_h  (A/B row-packed via base_partition 0/64)
    prAB = exp(sAB/8)              (one ScalarE ACT per g, bf16 out)
    yA[65,512] += vaugA[:,g,:].T @ prAB[:, :512]   (ones col -> row 64 denom)
    yB[65,512] += vaugB[:,g,:].T @ prAB[:, 512:]
    + ~1 interleaved projection matmul of pair p+1 per g (fills exp-wait gaps,
      keeps PE dense so HAM stays at K=8/8)
  evacuate yA/yB to SBUF right after AV(15) (frees PSUM banks for next block;
  reciprocal + partition-broadcast + normalize run in background)
  out[i,:]  = sum_p yTs[p].T-slice @ WoT[p]  -> fp32 partial, DMA out
PSUM: sAB x2 bufs (4 banks) + yA + yB (2) + pj x2 (2) = 8 banks.
"""

import os
import ml_dtypes
import numpy as np

B, T, C = 4, 2048, 1024
NH, HS = 16, 64
N_CORES = 8

_CACHE = {}
LAST_RESULTS = {}


def _build_nc(debug_taps=False):
    import concourse.bass as bass
    import concourse.mybir as mybir
    import concourse.tile as tile
    from concourse import bacc

    F32 = mybir.dt.float32
    F16 = mybir.dt.bfloat16
    AF = mybir.ActivationFunctionType
    ALU = mybir.AluOpType

    nc = bacc.Bacc("TRN2", target_bir_lowering=False, debug=False, num_devices=N_CORES)

    xT = nc.dram_tensor("xT", [C, T], F16, kind="ExternalInput").ap()
    wkT = nc.dram_tensor("wkT", [C, 512], F16, kind="ExternalInput").ap()
    wqT = nc.dram_tensor("wqT", [C, 512], F16, kind="ExternalInput").ap()
    wvT = nc.dram_tensor("wvT", [C, 512], F16, kind="ExternalInput").ap()
    woT = nc.dram_tensor("woT", [512, C], F16, kind="ExternalInput").ap()
    out = nc.dram_tensor("out", [T, C], F16, kind="ExternalOutput").ap()

    NPAIR = 4   # head pairs per core
    NIB = 4     # query blocks of 512
    NG = 16     # key tiles of 128

    with tile.TileContext(nc) as tc:
        with tc.tile_pool(name="sb", bufs=1) as sb, \
             tc.tile_pool(name="ps", bufs=1, space="PSUM") as ps:
            xTs = [sb.tile([128, T], F16, tag=f"xT{c}", name=f"xT{c}") for c in range(8)]
            wvh = [sb.tile([128, 512], F16, tag=f"wv{c}", name=f"wv{c}") for c in range(8)]
            wkh = [sb.tile([128, 512], F16, tag=f"wk{c}", name=f"wk{c}") for c in range(8)]
            wqh = [sb.tile([128, 512], F16, tag=f"wq{c}", name=f"wq{c}") for c in range(8)]
            wos = [sb.tile([128, C], F16, tag=f"wo{p}", name=f"wo{p}") for p in range(NPAIR)]
            kT = [sb.tile([128, T], F16, tag=f"kT{p}", name=f"kT{p}") for p in range(NPAIR)]
            qT = [sb.tile([128, T], F16, tag=f"qT{p}", name=f"qT{p}") for p in range(NPAIR)]
            yTs = [sb.tile([128, T], F16, tag=f"yT{p}", name=f"yT{p}") for p in range(NPAIR)]
            vaug = [sb.tile([128, NG, 65], F16, tag=f"va{h}", name=f"va{h}")
                    for h in range(8)]
            ones16 = sb.tile([128, NG], F16, tag="ones16", name="ones16")
            nc.vector.memset(ones16[:], 1.0)
            for h in range(8):
                nc.sync.dma_start(vaug[h][:, :, 64:65], ones16[:].unsqueeze(2))
            # k/q weights first so the pair-0 projections pipeline with the
            # (bigger) x transfer right behind them on the same queue.
            for c in range(8):
                nc.sync.dma_start(wkh[c][:], wkT[c * 128:(c + 1) * 128, :])
                nc.sync.dma_start(wqh[c][:], wqT[c * 128:(c + 1) * 128, :])
            # x rides the ScalarE HWDGE queue in parallel with the weights on
            # the sync queue -- one queue alone caps well below HBM bandwidth
            for c in range(8):
                nc.scalar.dma_start(xTs[c][:], xT[c * 128:(c + 1) * 128, :])
            for c in range(8):
                nc.sync.dma_start(wvh[c][:], wvT[c * 128:(c + 1) * 128, :])
            for p in range(NPAIR):
                nc.sync.dma_start(wos[p][:], woT[p * 128:(p + 1) * 128, :])

            # PE warmup burst: dep-free matmuls on a constant tile keep the
            # PE busy through the HAM SHORT window while the input DMAs land,
            # so real work starts at K=8/8 instead of half clock.
            wu = sb.tile([128, 512], F16, tag="wu", name="wu")
            nc.vector.memset(wu[:], 0.5)
            for i in range(36):
                wp = ps.tile([128, 512], F32, tag="pj", bufs=2, name=f"wu{i}")
                nc.tensor.matmul(wp[:], wu[:, 0:128], wu[:],
                                 start=True, stop=True)

            def kq_chunks(p):
                """Yield thunks; each emits one psum chunk (8 MMs + copy) of
                pair p's kT or qT. Interleaved k/q per column block so the
                first scores matmul is unblocked after two chunks."""
                for nn in range(4):
                    for wsb, wnm, dst in ((wkh, "wk", kT[p]), (wqh, "wq", qT[p])):
                        def chunk(wsb=wsb, wnm=wnm, dst=dst, nn=nn, p=p):
                            kp = ps.tile([128, 512], F32, tag="pj", bufs=2,
                                         name=f"pj_{wnm}{p}_{nn}")
                            for c in range(8):
                                nc.tensor.matmul(
                                    kp[:], wsb[c][:, p * 128:(p + 1) * 128],
                                    xTs[c][:, nn * 512:(nn + 1) * 512],
                                    start=(c == 0), stop=(c == 7))
                            nc.vector.tensor_copy(dst[:, nn * 512:(nn + 1) * 512],
                                                  kp[:])
                        yield chunk

            # ---- v = x @ Wv.T -> vaug SBUF tiles (fp16), no DRAM scratch ----
            def v_chunk(tt):
                vp = ps.tile([128, 512], F32, tag="pj", bufs=2, name=f"vps{tt}")
                for c in range(8):
                    nc.tensor.matmul(vp[:], xTs[c][:, tt * 128:(tt + 1) * 128],
                                     wvh[c][:], start=(c == 0), stop=(c == 7))
                for h in range(8):
                    nc.vector.tensor_copy(vaug[h][:, tt, 0:64],
                                          vp[:, h * 64:(h + 1) * 64])

            # pair-0 k/q first (unblocks attention); v tiles are spread
            # one-per-g inside pair-0's first query block.
            for ch in kq_chunks(0):
                ch()

            # ---- final projection, emitted in halves so the first half
            # overlaps the tail of pair-3 attention ----
            def wo_tiles(its):
                for it in its:
                    ob = sb.tile([128, C], F16, tag="ob", bufs=2, name=f"ob{it}")
                    for n2 in range(2):
                        fp_ = ps.tile([128, 512], F32, tag="pj", bufs=2,
                                      name=f"fin{it}_{n2}")
                        for pp in range(NPAIR):
                            nc.tensor.matmul(fp_[:],
                                             yTs[pp][:, it * 128:(it + 1) * 128],
                                             wos[pp][:, n2 * 512:(n2 + 1) * 512],
                                             start=(pp == 0), stop=(pp == 3))
                        nc.vector.tensor_copy(ob[:, n2 * 512:(n2 + 1) * 512],
                                              fp_[:])
                    # output goes out on the GpSimd SWDGE queue so it never
                    # delays the recip-broadcast chain on the sync queue
                    nc.gpsimd.dma_start(out[it * 128:(it + 1) * 128, :], ob[:])

            # ---- attention: 4 pairs x 4 query-blocks x 16 key-tiles ----
            pending_norm = []
            for p in range(NPAIR):
                hA, hB = 2 * p, 2 * p + 1
                spread = list(kq_chunks(p + 1)) if p + 1 < NPAIR else []
                for ib in range(NIB):
                    io = ib * 512
                    yA = ps.tile([65, 512], F32, tag="yA", name=f"yA{p}_{ib}")
                    yB = ps.tile([65, 512], F32, tag="yB", name=f"yB{p}_{ib}")
                    prs = {}
                    for g in range(NG + 1):
                        if g < NG:
                            sAB = ps.tile([128, 1024], F32, tag="sAB", bufs=2,
                                          name=f"s{p}_{ib}_{g}")
                            nc.tensor.matmul(sAB[:, 0:512],
                                             kT[p][0:64, g * 128:(g + 1) * 128],
                                             qT[p][0:64, io:io + 512],
                                             start=True, stop=True)
                            nc.tensor.matmul(sAB[:, 512:1024],
                                             kT[p][64:128, g * 128:(g + 1) * 128],
                                             qT[p][64:128, io:io + 512],
                                             start=True, stop=True)
                            pr = sb.tile([128, 1024], F16, tag="pr", bufs=3,
                                         name=f"pr{p}_{ib}_{g}")
                            nc.scalar.activation(pr[:], sAB[:], AF.Exp, scale=0.125)
                            prs[g] = pr
                        # fill the exp-wait PE gaps: v tiles (pair 0, first
                        # block -- tile g must exist before AV consumes it),
                        # then next pair's k/q projections, then wo tiles
                        # during the last pair.
                        # flush the previous block's normalize multiplies a
                        # few iterations in: their broadcast DMA chain has
                        # landed by then, so the DVE queue never waits on it.
                        if g == 6:
                            for fn in pending_norm:
                                fn()
                            pending_norm = []
                        if p == 0 and ib == 0:
                            if g < NG:
                                v_chunk(g)
                        elif spread and g % 6 == 3:
                            spread.pop(0)()
                        if (p == NPAIR - 1 and ib >= 1 and g >= 9
                                and g % 2 == 1):
                            wo_tiles([(ib - 1) * 4 + (g - 9) // 2])
                        if g > 0:
                            gg = g - 1
                            nc.tensor.matmul(yA[:], vaug[hA][:, gg, :],
                                             prs[gg][:, 0:512],
                                             start=(gg == 0), stop=(gg == 15))
                            nc.tensor.matmul(yB[:], vaug[hB][:, gg, :],
                                             prs[gg][:, 512:1024],
                                             start=(gg == 0), stop=(gg == 15))
                            if gg > 1:
                                del prs[gg - 1]

                    # evacuate PSUM fast: one fp32 copy per head frees the
                    # banks; both heads share one denominator pipeline.
                    stAB = sb.tile([65, 1024], F32, tag="st", bufs=3,
                                   name=f"st{p}_{ib}")
                    nc.vector.tensor_copy(stAB[:, 0:512], yA[:])
                    nc.vector.tensor_copy(stAB[:, 512:1024], yB[:])
                    # spray the joint denominator row across 128 partitions
                    # (cheap DVE reciprocal), unspray, then one split
                    # partition-broadcast: head A recips -> rows 0-63,
                    # head B -> rows 64-127.
                    rsp = sb.tile([128, 8], F32, tag="rsp", bufs=2,
                                  name=f"rsp{p}_{ib}")
                    nc.sync.dma_start(rsp[:], stAB[64:65, :])
                    rrec = sb.tile([128, 8], F32, tag="rrec", bufs=2,
                                   name=f"rrec{p}_{ib}")
                    nc.vector.reciprocal(rrec[:], rsp[:])
                    rrow = sb.tile([1, 2, 512], F32, tag="rrow", bufs=2,
                                   name=f"rrow{p}_{ib}")
                    nc.sync.dma_start(rrow[:], rrec[:])
                    bcAB = sb.tile([64, 1024], F32, tag="bc", bufs=2,
                                   name=f"bc{p}_{ib}")
                    nc.sync.dma_start(
                        bcAB[:, 0:512],
                        rrow[:, 0, :].unsqueeze(1).to_broadcast((1, 64, 512)))
                    nc.sync.dma_start(
                        bcAB[:, 512:1024],
                        rrow[:, 1, :].unsqueeze(1).to_broadcast((1, 64, 512)))

                    def norm(p=p, io=io, stAB=stAB, bcAB=bcAB):
                        nc.vector.tensor_tensor(
                            out=yTs[p][0:64, io:io + 512],
                            in0=stAB[0:64, 0:512], in1=bcAB[:, 0:512],
                            op=ALU.mult)
                        nc.vector.tensor_tensor(
                            out=yTs[p][64:128, io:io + 512],
                            in0=stAB[0:64, 512:1024], in1=bcAB[:, 512:1024],
                            op=ALU.mult)
                    pending_norm.append(norm)

            for fn in pending_norm:
                fn()
            wo_tiles(range(12, 16))

    nc.compile()
    return nc


def _get_nc():
    if "nc" not in _CACHE:
        _CACHE["nc"] = _build_nc()
    return _CACHE["nc"]


def _make_in_maps(x, Wk, Wq, Wv, Wo):
    wkT = np.ascontiguousarray(Wk.T).astype(ml_dtypes.bfloat16)
    wqT = np.ascontiguousarray(Wq.T).astype(ml_dtypes.bfloat16)
    wvT = np.ascontiguousarray(Wv.T).astype(ml_dtypes.bfloat16)
    woT = np.ascontiguousarray(Wo.T).astype(ml_dtypes.bfloat16)
    in_maps = []
    for core in range(N_CORES):
        b, hh = core // 2, core % 2
        sl = slice(hh * 512, (hh + 1) * 512)
        xb = np.asarray(x[b], dtype=np.float32)
        in_maps.append({
            "xT": np.ascontiguousarray(xb.T).astype(ml_dtypes.bfloat16),
            "wkT": np.ascontiguousarray(wkT[:, sl]),
            "wqT": np.ascontiguousarray(wqT[:, sl]),
            "wvT": np.ascontiguousarray(wvT[:, sl]),
            "woT": np.ascontiguousarray(woT[sl, :]),
        })
    return in_maps


def _install_ntff_hook_shim():
    import sys, types
    try:
        from antenv.axon_hooks import get_axon_ntff_profile_hook  # noqa
        return True
    except ImportError:
        pass
    try:
        sys.path.insert(0, "/root/.axon_site")
        from trn_agent_boot.trn_boot import _ntff_profile_via_ctypes
        hook = _ntff_profile_via_ctypes("/opt/axon/libaxon_pjrt.so")
        if hook is None:
            return False
        mod = types.ModuleType("antenv.axon_hooks")
        mod._hook = hook
        mod.get_axon_ntff_profile_hook = lambda: mod._hook
        mod.set_axon_ntff_profile_hook = lambda h: setattr(mod, "_hook", h)
        sys.modules["antenv.axon_hooks"] = mod
        import antenv
        antenv.axon_hooks = mod
        return True
    except Exception:
        return False


def kernel(x, Wk, Wq, Wv, Wo):
    from concourse.bass_utils import run_bass_kernel_spmd

    nc = _get_nc()
    in_maps = _make_in_maps(x, Wk, Wq, Wv, Wo)
    trace = bool(int(os.environ.get("ATT_TRACE", "0")))
    if trace and not _install_ntff_hook_shim():
        trace = False
    res = run_bass_kernel_spmd(nc, in_maps, core_ids=list(range(N_CORES)),
                               trace=trace)
    LAST_RESULTS["exec_time_ns"] = res.exec_time_ns
    LAST_RESULTS["res"] = res
    full = np.empty((B, T, C), dtype=np.float32)
    for b in range(B):
        full[b] = (res.results[2 * b]["out"].astype(np.float32)
                   + res.results[2 * b + 1]["out"].astype(np.float32))
    return full


# revision 36
# speedup vs baseline: 1.0418x; 1.0418x over previous
"""Multi-head attention (B=4, T=2048, C=1024, 16 heads, no mask) on 8 TRN2 cores.

Sharding: batch DP x head TP per the hint. Core c handles batch b=c//2 and
heads 8*(c%2)..8*(c%2)+7 (4 head-pairs) over ALL 2048 queries. Each core
computes a PARTIAL output (its 8 heads through its Wo row-slice); the host
sums the two partials per batch (free w.r.t. HW exec time). No duplicated
K/V work, no DRAM scratch: q/k/v live entirely in SBUF.

All matmul operands bf16 (fp32 PSUM accumulate). Device dataflow per core:
  v[t,d]    = xT.T @ WvT(slice)    -> SBUF aug tiles [128j, 16g, 65] (ones col)
  kT[p]     = WkT(chunk).T @ xT    qT[p] = WqT(chunk).T @ xT   (per pair p)
  per pair p, query-block ib (512), key-tile g (128):
    sAB[j, 2*512] = kT_h.T @ qT_h  (A/B row-packed via base_partition 0/64)
    prAB = exp(sAB/8)              (one ScalarE ACT per g, bf16 out)
    yA[65,512] += vaugA[:,g,:].T @ prAB[:, :512]   (ones col -> row 64 denom)
    yB[65,512] += vaugB[:,g,:].T @ prAB[:, 512:]
    + ~1 interleaved projection matmul of pair p+1 per g (fills exp-wait gaps,
      keeps PE dense so HAM stays at K=8/8)
  evacuate yA/yB to SBUF right after AV(15) (frees PSUM banks for next block;
  reciprocal + partition-broadcast + normalize run in background)
  out[i,:]  = sum_p yTs[p].T-slice @ WoT[p]  -> fp32 partial, DMA out
PSUM: sAB x2 bufs (4 banks) + yA + yB (2) + pj x2 (2) = 8 banks.
"""

import os
import ml_dtypes
import numpy as np

B, T, C = 4, 2048, 1024
NH, HS = 16, 64
N_CORES = 8

_CACHE = {}
LAST_RESULTS = {}


def _build_nc(debug_taps=False):
    import concourse.bass as bass
    import concourse.mybir as mybir
    import concourse.tile as tile
    from concourse import bacc

    F32 = mybir.dt.float32
    F16 = mybir.dt.bfloat16
    AF = mybir.ActivationFunctionType
    ALU = mybir.AluOpType

    nc = bacc.Bacc("TRN2", target_bir_lowering=False, debug=False, num_devices=N_CORES)

    xT = nc.dram_tensor("xT", [C, T], F16, kind="ExternalInput").ap()
    wkT = nc.dram_tensor("wkT", [C, 512], F16, kind="ExternalInput").ap()
    wqT = nc.dram_tensor("wqT", [C, 512], F16, kind="ExternalInput").ap()
    wvT = nc.dram_tensor("wvT", [C, 512], F16, kind="ExternalInput").ap()
    woT = nc.dram_tensor("woT", [512, C], F16, kind="ExternalInput").ap()
    out = nc.dram_tensor("out", [T, C], F16, kind="ExternalOutput").ap()

    NPAIR = 4   # head pairs per core
    NIB = 4     # query blocks of 512
    NG = 16     # key tiles of 128

    with tile.TileContext(nc) as tc:
        with tc.tile_pool(name="sb", bufs=1) as sb, \
             tc.tile_pool(name="ps", bufs=1, space="PSUM") as ps:
            xTs = [sb.tile([128, T], F16, tag=f"xT{c}", name=f"xT{c}") for c in range(8)]
            wvh = [sb.tile([128, 512], F16, tag=f"wv{c}", name=f"wv{c}") for c in range(8)]
            wkh = [sb.tile([128, 512], F16, tag=f"wk{c}", name=f"wk{c}") for c in range(8)]
            wqh = [sb.tile([128, 512], F16, tag=f"wq{c}", name=f"wq{c}") for c in range(8)]
            wos = [sb.tile([128, C], F16, tag=f"wo{p}", name=f"wo{p}") for p in range(NPAIR)]
            kT = [sb.tile([128, T], F16, tag=f"kT{p}", name=f"kT{p}") for p in range(NPAIR)]
            qT = [sb.tile([128, T], F16, tag=f"qT{p}", name=f"qT{p}") for p in range(NPAIR)]
            yTs = [sb.tile([128, T], F16, tag=f"yT{p}", name=f"yT{p}") for p in range(NPAIR)]
            vaug = [sb.tile([128, NG, 65], F16, tag=f"va{h}", name=f"va{h}")
                    for h in range(8)]
            ones16 = sb.tile([128, NG], F16, tag="ones16", name="ones16")
            nc.vector.memset(ones16[:], 1.0)
            ones64 = sb.tile([1, 64], F16, tag="ones64", name="ones64")
            nc.vector.memset(ones64[:], 1.0)
            for h in range(8):
                nc.sync.dma_start(vaug[h][:, :, 64:65], ones16[:].unsqueeze(2))
            # k/q weights first so the pair-0 projections pipeline with the
            # (bigger) x transfer right behind them on the same queue.
            for c in range(8):
                nc.sync.dma_start(wkh[c][:], wkT[c * 128:(c + 1) * 128, :])
                nc.sync.dma_start(wqh[c][:], wqT[c * 128:(c + 1) * 128, :])
            # x rides the ScalarE HWDGE queue in parallel with the weights on
            # the sync queue -- one queue alone caps well below HBM bandwidth
            for c in range(8):
                nc.scalar.dma_start(xTs[c][:], xT[c * 128:(c + 1) * 128, :])
            for c in range(8):
                nc.scalar.dma_start(wvh[c][:], wvT[c * 128:(c + 1) * 128, :])
            for p in range(NPAIR):
                nc.scalar.dma_start(wos[p][:], woT[p * 128:(p + 1) * 128, :])

            # PE warmup burst: dep-free matmuls on a constant tile keep the
            # PE busy through the HAM SHORT window while the input DMAs land,
            # so real work starts at K=8/8 instead of half clock.
            wu = sb.tile([128, 512], F16, tag="wu", name="wu")
            nc.vector.memset(wu[:], 0.5)
            for i in range(36):
                wp = ps.tile([128, 512], F32, tag="pj", bufs=2, name=f"wu{i}")
                nc.tensor.matmul(wp[:], wu[:, 0:128], wu[:],
                                 start=True, stop=True)

            def kq_chunks(p):
                """Yield thunks; each emits one psum chunk (8 MMs + copy) of
                pair p's kT or qT. Interleaved k/q per column block so the
                first scores matmul is unblocked after two chunks."""
                for nn in range(4):
                    for wsb, wnm, dst in ((wkh, "wk", kT[p]), (wqh, "wq", qT[p])):
                        def chunk(wsb=wsb, wnm=wnm, dst=dst, nn=nn, p=p):
                            kp = ps.tile([128, 512], F32, tag="pj", bufs=2,
                                         name=f"pj_{wnm}{p}_{nn}")
                            for c in range(8):
                                nc.tensor.matmul(
                                    kp[:], wsb[c][:, p * 128:(p + 1) * 128],
                                    xTs[c][:, nn * 512:(nn + 1) * 512],
                                    start=(c == 0), stop=(c == 7))
                            nc.vector.tensor_copy(dst[:, nn * 512:(nn + 1) * 512],
                                                  kp[:])
                        yield chunk

            # ---- v = x @ Wv.T -> vaug SBUF tiles (fp16), no DRAM scratch ----
            def v_chunk(tt):
                vp = ps.tile([128, 512], F32, tag="pj", bufs=2, name=f"vps{tt}")
                for c in range(8):
                    nc.tensor.matmul(vp[:], xTs[c][:, tt * 128:(tt + 1) * 128],
                                     wvh[c][:], start=(c == 0), stop=(c == 7))
                for h in range(8):
                    nc.vector.tensor_copy(vaug[h][:, tt, 0:64],
                                          vp[:, h * 64:(h + 1) * 64])

            # pair-0 k/q first (unblocks attention); v tiles are spread
            # one-per-g inside pair-0's first query block.
            for ch in kq_chunks(0):
                ch()

            # ---- final projection, emitted in halves so the first half
            # overlaps the tail of pair-3 attention ----
            def wo_tiles(its):
                for it in its:
                    ob = sb.tile([128, C], F16, tag="ob", bufs=2, name=f"ob{it}")
                    for n2 in range(2):
                        fp_ = ps.tile([128, 512], F32, tag="pj", bufs=2,
                                      name=f"fin{it}_{n2}")
                        for pp in range(NPAIR):
                            nc.tensor.matmul(fp_[:],
                                             yTs[pp][:, it * 128:(it + 1) * 128],
                                             wos[pp][:, n2 * 512:(n2 + 1) * 512],
                                             start=(pp == 0), stop=(pp == 3))
                        nc.vector.tensor_copy(ob[:, n2 * 512:(n2 + 1) * 512],
                                              fp_[:])
                    # output goes out on the GpSimd SWDGE queue so it never
                    # delays the recip-broadcast chain on the sync queue
                    nc.gpsimd.dma_start(out[it * 128:(it + 1) * 128, :], ob[:])

            # ---- attention: 4 pairs x 4 query-blocks x 16 key-tiles ----
            pending_norm = []
            for p in range(NPAIR):
                hA, hB = 2 * p, 2 * p + 1
                spread = list(kq_chunks(p + 1)) if p + 1 < NPAIR else []
                for ib in range(NIB):
                    io = ib * 512
                    yA = ps.tile([65, 512], F32, tag="yA", name=f"yA{p}_{ib}")
                    yB = ps.tile([65, 512], F32, tag="yB", name=f"yB{p}_{ib}")
                    prs = {}
                    for g in range(NG + 1):
                        if g < NG:
                            sAB = ps.tile([128, 1024], F32, tag="sAB", bufs=2,
                                          name=f"s{p}_{ib}_{g}")
                            nc.tensor.matmul(sAB[:, 0:512],
                                             kT[p][0:64, g * 128:(g + 1) * 128],
                                             qT[p][0:64, io:io + 512],
                                             start=True, stop=True)
                            nc.tensor.matmul(sAB[:, 512:1024],
                                             kT[p][64:128, g * 128:(g + 1) * 128],
                                             qT[p][64:128, io:io + 512],
                                             start=True, stop=True)
                            pr = sb.tile([128, 1024], F16, tag="pr", bufs=3,
                                         name=f"pr{p}_{ib}_{g}")
                            nc.scalar.activation(pr[:], sAB[:], AF.Exp, scale=0.125)
                            prs[g] = pr
                        # fill the exp-wait PE gaps: v tiles (pair 0, first
                        # block -- tile g must exist before AV consumes it),
                        # then next pair's k/q projections, then wo tiles
                        # during the last pair.
                        # flush the previous block's normalize multiplies a
                        # few iterations in: their broadcast DMA chain has
                        # landed by then, so the DVE queue never waits on it.
                        if g == 6:
                            for fn in pending_norm:
                                fn()
                            pending_norm = []
                        if p == 0 and ib == 0:
                            if g < NG:
                                v_chunk(g)
                        elif spread and g % 6 == 3:
                            spread.pop(0)()
                        if (p == NPAIR - 1 and ib >= 1 and g >= 9
                                and g % 2 == 1):
                            wo_tiles([(ib - 1) * 4 + (g - 9) // 2])
                        if g > 0:
                            gg = g - 1
                            nc.tensor.matmul(yA[:], vaug[hA][:, gg, :],
                                             prs[gg][:, 0:512],
                                             start=(gg == 0), stop=(gg == 15))
                            nc.tensor.matmul(yB[:], vaug[hB][:, gg, :],
                                             prs[gg][:, 512:1024],
                                             start=(gg == 0), stop=(gg == 15))
                            if gg > 1:
                                del prs[gg - 1]

                    # evacuate PSUM fast: one fp32 copy per head frees the
                    # banks; both heads share one denominator pipeline.
                    stAB = sb.tile([65, 1024], F32, tag="st", bufs=3,
                                   name=f"st{p}_{ib}")
                    nc.vector.tensor_copy(stAB[:, 0:512], yA[:])
                    nc.vector.tensor_copy(stAB[:, 512:1024], yB[:])
                    # spray the joint denominator row across 128 partitions
                    # (cheap DVE reciprocal), unspray, then one split
                    # partition-broadcast: head A recips -> rows 0-63,
                    # head B -> rows 64-127.
                    rsp = sb.tile([128, 8], F32, tag="rsp", bufs=2,
                                  name=f"rsp{p}_{ib}")
                    nc.sync.dma_start(rsp[:], stAB[64:65, :])
                    rrec = sb.tile([128, 8], F32, tag="rrec", bufs=2,
                                   name=f"rrec{p}_{ib}")
                    nc.vector.reciprocal(rrec[:], rsp[:])
                    rrecb = sb.tile([128, 8], F16, tag="rrecb", bufs=2,
                                    name=f"rrecb{p}_{ib}")
                    nc.vector.tensor_copy(rrecb[:], rrec[:])
                    rrow = sb.tile([1, 2, 512], F16, tag="rrow", bufs=2,
                                   name=f"rrow{p}_{ib}")
                    nc.sync.dma_start(rrow[:], rrecb[:])

                    def norm(p=p, io=io, stAB=stAB, rrow=rrow):
                        # partition-broadcast of the recip rows on the PE (a
                        # K=1 ones matmul) -- a DMA broadcast takes ~9us of
                        # same-source packet reads, the matmul ~0.2us.
                        for h2 in range(2):
                            bcp = ps.tile([128, 512], F32, tag="pj", bufs=2,
                                          name=f"bcp{p}_{io}_{h2}")
                            nc.tensor.matmul(bcp[0:64, :], ones64[:],
                                             rrow[:, h2, :],
                                             start=True, stop=True)
                            nc.vector.tensor_tensor(
                                out=yTs[p][h2 * 64:(h2 + 1) * 64, io:io + 512],
                                in0=stAB[0:64, h2 * 512:(h2 + 1) * 512],
                                in1=bcp[0:64, :], op=ALU.mult)
                    pending_norm.append(norm)

            for fn in pending_norm:
                fn()
            wo_tiles(range(12, 16))

    nc.compile()
    return nc


def _get_nc():
    if "nc" not in _CACHE:
        _CACHE["nc"] = _build_nc()
    return _CACHE["nc"]


def _make_in_maps(x, Wk, Wq, Wv, Wo):
    wkT = np.ascontiguousarray(Wk.T).astype(ml_dtypes.bfloat16)
    wqT = np.ascontiguousarray(Wq.T).astype(ml_dtypes.bfloat16)
    wvT = np.ascontiguousarray(Wv.T).astype(ml_dtypes.bfloat16)
    woT = np.ascontiguousarray(Wo.T).astype(ml_dtypes.bfloat16)
    in_maps = []
    for core in range(N_CORES):
        b, hh = core // 2, core % 2
        sl = slice(hh * 512, (hh + 1) * 512)
        xb = np.asarray(x[b], dtype=np.float32)
        in_maps.append({
            "xT": np.ascontiguousarray(xb.T).astype(ml_dtypes.bfloat16),
            "wkT": np.ascontiguousarray(wkT[:, sl]),
            "wqT": np.ascontiguousarray(wqT[:, sl]),
            "wvT": np.ascontiguousarray(wvT[:, sl]),
            "woT": np.ascontiguousarray(woT[sl, :]),
        })
    return in_maps


def _install_ntff_hook_shim():
    import sys, types
    try:
        from antenv.axon_hooks import get_axon_ntff_profile_hook  # noqa
        return True
    except ImportError:
        pass
    try:
        sys.path.insert(0, "/root/.axon_site")
        from trn_agent_boot.trn_boot import _ntff_profile_via_ctypes
        hook = _ntff_profile_via_ctypes("/opt/axon/libaxon_pjrt.so")
        if hook is None:
            return False
        mod = types.ModuleType("antenv.axon_hooks")
        mod._hook = hook
        mod.get_axon_ntff_profile_hook = lambda: mod._hook
        mod.set_axon_ntff_profile_hook = lambda h: setattr(mod, "_hook", h)
        sys.modules["antenv.axon_hooks"] = mod
        import antenv
        antenv.axon_hooks = mod
        return True
    except Exception:
        return False


def kernel(x, Wk, Wq, Wv, Wo):
    from concourse.bass_utils import run_bass_kernel_spmd

    nc = _get_nc()
    in_maps = _make_in_maps(x, Wk, Wq, Wv, Wo)
    trace = bool(int(os.environ.get("ATT_TRACE", "0")))
    if trace and not _install_ntff_hook_shim():
        trace = False
    res = run_bass_kernel_spmd(nc, in_maps, core_ids=list(range(N_CORES)),
                               trace=trace)
    LAST_RESULTS["exec_time_ns"] = res.exec_time_ns
    LAST_RESULTS["res"] = res
    full = np.empty((B, T, C), dtype=np.float32)
    for b in range(B):
        full[b] = (res.results[2 * b]["out"].astype(np.float32)
                   + res.results[2 * b + 1]["out"].astype(np.float32))
    return full
